# revision 43
# baseline (speedup 1.0000x reference)
"""Fused dual-branch attention kernel for one TRN2 chip (8 NeuronCores).

Problem: x:[4,1024,1024], qkv_w:[3072,1024], proj_w:[1024,1024], proj_b:[1024],
attn_mask:[2,1,1024,1024].  Reference computes two attention branches sharing
the qkv/proj weights:
  x_ori = proj(attend(q, k, v, mask0)),  x_v = proj(attend(v, v, v, mask1))

Sharding: 8 cores = (2 branches x 4 batches), zero communication.  Every core
runs the SAME graph; branch differences are folded into the per-core weight
data (branch-1 cores get [v_w*s | v_w | v_w] as their "qkv" weight stack) and
the per-core mask data.  The softmax scale folds into the query weights; the
additive mask folds in multiplicatively as exp(mask) (no max-subtraction:
logits are bounded ~+-8 for this distribution, exp stays in fp32 range).

Matmul cost on the PE is (output free size) x (cycles/row), independent of
the contraction partition count, so every matmul is arranged to stream its
SMALL dimension:

  ST[m,nq]   = sum_d BT[d,m] * AT[d,nq]        (K=64, 1024-col stream, psum)
  PT[m,nq]   = exp(ST) * em[m,nq]              (ACT exp -> DVE/GpSimd mult)
  PV[nq,d]   = sum_m PT[m,nq-tile]^T V[m,d]    (K=128, 64-col stream!)
  den[nq,1]  = sum_m PT[m,nq-tile]^T ones      (1-col stream, own psum bank)
  A[nq,c]    = PV * recip(den)                 (den is PER-PARTITION here, so
                                                normalize is a plain DVE
                                                tensor_scalar -- no cross-
                                                partition broadcast)
  AT[c,nq]   = PE-transpose(A)                 (128-col streams, cheap)
  yT         = pwT^T @ AT + b

This halves the PV stream cost vs the [d,nq] layout (4160 vs 8192 cols/head)
and deletes the DRAM-bounce reciprocal broadcast of the old design.

Steady-state schedule per head h (steps m=0..7):
  each step: ST(h,m) -> ACT exp -> mask-mult (DVE, or GpSimd for m in 0,2,4)
  steps 0-3: PV+den t-groups 2m,2m+1 of head h-1 (t-major: all of P(h-1) is
             ready, and one pending psum group per bank at a time)
  step 4:    recip(h-1)  [4 steps before the den bank is written again ->
             the PE never waits on it]
  steps 0-4: normalize-evicts of head h-2, nq-tiles 3..7
  steps 5-7: normalize-evicts of head h-1, nq-tiles 0..2
  odd head ends: PE-transpose one finished head-pair into ot
A1 q/k projection halves and the A2 v-projection are paced in as PE filler
to keep the PE saturated while ACT grinds the exps.
"""

import numpy as np
import ml_dtypes

import concourse.bass as bass
from concourse import bacc
import concourse.tile as tile
import concourse.mybir as mybir
from contextlib import ExitStack

B, N, C, H, D, P, NF = 4, 1024, 1024, 16, 64, 128, 512
BF16 = mybir.dt.bfloat16
F32 = mybir.dt.float32
AF = mybir.ActivationFunctionType

_nc_cache = None


def _build(reps=1):
    nc = bacc.Bacc("TRN2", target_bir_lowering=False, debug=False, num_devices=8)
    xT = nc.declare_dram_parameter("xT", [C, N], BF16, isOutput=False)
    wT = nc.declare_dram_parameter("wT", [C, 3 * C], BF16, isOutput=False)
    em = nc.declare_dram_parameter("em", [N, N], BF16, isOutput=False)
    pwT = nc.declare_dram_parameter("pwT", [C, C], BF16, isOutput=False)
    pb = nc.declare_dram_parameter("pb", [C], F32, isOutput=False)
    ident = nc.declare_dram_parameter("ident", [P, P], F32, isOutput=False)
    out = nc.declare_dram_parameter("out", [C, N], F32, isOutput=True)

    with tile.TileContext(nc) as tc:
        for _ in range(reps):
            with ExitStack() as ctx:
                _body(tc, ctx, xT, wT, em, pwT, pb, ident, out)
    nc.compile()
    return nc


def _body(tc, ctx, xT, wT, em, pwT, pb, ident, out):
    nc = tc.nc

    pers = ctx.enter_context(tc.tile_pool(name="pers", bufs=1))
    work = ctx.enter_context(tc.tile_pool(name="work", bufs=1))
    psum = ctx.enter_context(tc.tile_pool(name="psum", bufs=1, space="PSUM"))

    def st_tile(nm):
        # ST scores / A1 / A2 / proj share the wide 2-bank slots (4 banks)
        return psum.tile([P, N], F32, name=nm, tag="st", bufs=2)

    # ---- input DMAs, interleaved per c-chunk in first-use order (the DMA
    # engines are a single pooled FIFO resource: emission order = service
    # order, so the first compute's operands must be emitted first) ----
    xw = ctx.enter_context(tc.tile_pool(name="xw", bufs=1))
    # ot (transposed attention out) sits below wc on the pool stack so wc can
    # be released at h==8 (LIFO) and the proj weights take its space
    mid = ctx.enter_context(tc.tile_pool(name="mid", bufs=1))
    ot = [mid.tile([P, N], BF16, name=f"ot{i}", tag=f"ot{i}") for i in range(8)]
    wcp_ctx = tc.tile_pool(name="wc", bufs=1)
    wcp = wcp_ctx.__enter__()
    x_t, w_t, wc_t = [], [], []
    for c in range(8):
        x = xw.tile([P, N], BF16, name=f"x{c}", tag=f"x{c}")
        w = xw.tile([P, 2 * C], BF16, name=f"w{c}", tag=f"w{c}")
        wc = wcp.tile([P, C], BF16, name=f"wc{c}", tag=f"wc{c}")
        # pre-loop A2(vf=0, m<4) + A1 kickstart (pairs 0, 1) come first
        nc.sync.dma_start(x[:, 0:NF], xT[c * P:(c + 1) * P, 0:NF])
        nc.sync.dma_start(wc[:, 0:NF], wT[c * P:(c + 1) * P, 2 * C:2 * C + NF])
        nc.sync.dma_start(w[:, 0:2 * P], wT[c * P:(c + 1) * P, 0:2 * P])
        nc.sync.dma_start(w[:, 8 * P:10 * P], wT[c * P:(c + 1) * P, 8 * P:10 * P])
        x_t.append(x); w_t.append(w); wc_t.append(wc)
    for c in range(8):
        nc.sync.dma_start(x_t[c][:, NF:N], xT[c * P:(c + 1) * P, NF:N])
    em_t = []
    for m in range(8):
        t = pers.tile([P, N], BF16, name=f"em{m}", tag=f"em{m}")
        nc.sync.dma_start(t[:], em[m * P:(m + 1) * P, :])
        em_t.append(t)
    for c in range(8):
        nc.sync.dma_start(w_t[c][:, 2 * P:8 * P], wT[c * P:(c + 1) * P, 2 * P:8 * P])
        nc.sync.dma_start(w_t[c][:, 10 * P:16 * P], wT[c * P:(c + 1) * P, 10 * P:16 * P])
    for c in range(8):
        nc.sync.dma_start(wc_t[c][:, NF:C], wT[c * P:(c + 1) * P, 2 * C + NF:3 * C])
    pb_t = pers.tile([P, 8], F32, name="pb", tag="pb")
    nc.sync.dma_start(pb_t[:], pb.rearrange("(t p) -> p t", p=P))
    id_t = pers.tile([P, P], F32, name="id", tag="id")
    nc.sync.dma_start(id_t[:], ident[:, :])

    ones_t = pers.tile([P, 4], BF16, name="ones", tag="ones")
    nc.vector.memset(ones_t[:], 1.0)
    abt = [pers.tile([P, N], BF16, name=f"abt{i}", tag=f"abt{i}") for i in range(16)]
    v_t = [pers.tile([P, N], BF16, name=f"v{m}", tag=f"v{m}") for m in range(8)]
    # normalized attention output (f32 so the PE-transpose staging can share
    # a psum bank with den), [nq-tile][:, ccslot*128 + (h%2)*64 + d];
    # ring of 4 head-pair column slots per nq tile
    an = [pers.tile([P, 4 * P], F32, name=f"an{t}", tag=f"an{t}") for t in range(8)]

    # one psum bank shared by the transpose staging (cols 0:384, f32) and the
    # softmax denominators (cols 384:392): den's t-major groups are all
    # stopped by the time a transpose's start=True clears the bank's
    # has_written bits, and stopped values survive until overwritten
    misc = psum.tile([P, 392], F32, name="misc", tag="misc", bufs=1)
    den8 = misc[:, 384:392]
    # dedicated 1-bank slot for fine-grained A1/A2 filler chunks: a half
    # accumulation group can stay open across steps without blocking the
    # ST double-buffer rotation
    qkps = psum.tile([P, NF], F32, name="qkps", tag="qk", bufs=1)

    # ---- PE building blocks ----
    def a1_mms(mt, nh, cs, ce):
        for c in range(cs, ce):
            nc.tensor.matmul(
                qkps[:, 0:NF],
                lhsT=w_t[c][:, mt * P:(mt + 1) * P],
                rhs=x_t[c][:, nh * NF:(nh + 1) * NF],
                start=(c == 0), stop=(c == 7),
            )

    def a2_mms(vf, m, cs, ce):
        for c in range(cs, ce):
            nc.tensor.matmul(
                qkps[:, 0:NF],
                lhsT=x_t[c][:, m * P:(m + 1) * P],
                rhs=wc_t[c][:, vf * NF:(vf + 1) * NF],
                start=(c == 0), stop=(c == 7),
            )

    def item_chunk(item, half):
        kind, a, b = item
        cs, ce = (0, 4) if half == 0 else (4, 8)
        if kind == "a1":
            a1_mms(a, b, cs, ce)
            if half == 1:
                nc.vector.tensor_copy(abt[a][:, b * NF:(b + 1) * NF], qkps[:, 0:NF])
        else:
            a2_mms(a, b, cs, ce)
            if half == 1:
                nc.vector.tensor_copy(v_t[b][:, a * NF:(a + 1) * NF], qkps[:, 0:NF])

    def a1_full(mt, nh):
        # full-group variant in the wide st slots (prologue / head 0 only)
        ps = st_tile(f"qk{mt}_{nh}")
        for c in range(8):
            nc.tensor.matmul(
                ps[:, 0:NF],
                lhsT=w_t[c][:, mt * P:(mt + 1) * P],
                rhs=x_t[c][:, nh * NF:(nh + 1) * NF],
                start=(c == 0), stop=(c == 7),
            )
        nc.vector.tensor_copy(abt[mt][:, nh * NF:(nh + 1) * NF], ps[:, 0:NF])

    def a2_full(vf, m):
        ps = st_tile(f"v{vf}_{m}")
        for c in range(8):
            nc.tensor.matmul(
                ps[:, 0:NF],
                lhsT=x_t[c][:, m * P:(m + 1) * P],
                rhs=wc_t[c][:, vf * NF:(vf + 1) * NF],
                start=(c == 0), stop=(c == 7),
            )
        nc.vector.tensor_copy(v_t[m][:, vf * NF:(vf + 1) * NF], ps[:, 0:NF])

    # filler ITEMS: one A1 half-tile or A2 half-v-tile each (8 matmuls),
    # emitted as two 4-matmul chunks at consecutive steps so the PE stream
    # stays fine-grained.  Item k of a head occupies steps (2+2k, 3+2k).
    # Deadlines: pair p of A1 feeds heads 2p..2p+1; A2 vf=1 feeds heads 8+.
    A1I = lambda p: [("a1", p, 0), ("a1", p, 1), ("a1", 8 + p, 0), ("a1", 8 + p, 1)]
    p = {q: A1I(q) for q in range(1, 8)}
    a2i = [("a2", 1, m) for m in range(8)]
    items_by_head = {
        1: p[1][2:],
        2: p[2][0:3], 3: [p[2][3], p[3][0], p[3][1]],
        4: [p[3][2], p[3][3], a2i[0]],
        5: [a2i[1], a2i[2], p[4][0]],
        6: [a2i[3], a2i[4], p[4][1]],
        7: [a2i[5], p[4][2], p[4][3]],
        8: [a2i[6], a2i[7], p[5][0]],
        9: p[5][1:4],
        10: p[6][0:3], 11: [p[6][3], p[7][0], p[7][1]],
        12: p[7][2:4],
    }

    # transposes of head-pair cc at end of head 2cc+3 (its last normalize-
    # evict lands at step (2cc+3, 5))
    tp_sched = {2 * cc + 3: cc for cc in range(7)}

    pw_t = []

    def emit_transposes(cc):
        slot = cc % 4
        for wv, ts in enumerate((range(0, 3), range(3, 6), range(6, 8))):
            for t in ts:
                nc.tensor.transpose(
                    misc[:, (t - 3 * wv) * P:(t - 3 * wv + 1) * P],
                    an[t][:, slot * P:(slot + 1) * P],
                    id_t[:],
                )
            w = len(ts) * P
            with nc.allow_low_precision(reason="attn out bf16 as baseline"):
                nc.vector.tensor_copy(ot[cc][:, 3 * wv * P:3 * wv * P + w],
                                      misc[:, 0:w])

    def norm_evict(he, t):
        slot = (he // 2) % 4
        nc.vector.tensor_scalar_mul(
            an[t][:, slot * P + (he % 2) * 64:slot * P + (he % 2) * 64 + 64],
            pv_t[he][:, t * 64:(t + 1) * 64],
            rc_t[he][:, t:t + 1],
        )

    # ---- prologue: A2(vf=0) m=0..3 all need only the FIRST x/wc halves, so
    # interleave their matmuls per c-chunk to ride the incoming DMA stream
    # (4 pending groups spread across the 4 banks of two st slots) ----
    a2ps = []
    for g in range(2):
        ps = st_tile(f"v0p{g}")
        a2ps += [ps[:, 0:NF], ps[:, NF:N]]
    for c in range(8):
        for g in range(4):
            nc.tensor.matmul(
                a2ps[g],
                lhsT=x_t[c][:, g * P:(g + 1) * P],
                rhs=wc_t[c][:, 0:NF],
                start=(c == 0), stop=(c == 7),
            )
    for g in range(4):
        nc.vector.tensor_copy(v_t[g][:, 0:NF], a2ps[g])
    for mt, nh in ((0, 0), (0, 1), (8, 0), (8, 1), (1, 0), (1, 1)):
        a1_full(mt, nh)

    pts = [[None] * 8, [None] * 8]
    pv_t, rc_t = {}, {}
    for h in range(17):
        hl = h - 1
        if h < 16:
            tl, ro = h // 2, (h % 2) * 64
            q_ap, k_ap = abt[tl], abt[8 + tl]
        if hl >= 0:
            pv_t[hl] = psum.tile([P, NF], F32, name=f"pv{hl}", tag="pv", bufs=2)
        for m in range(8):
            if h < 16:
                ps = st_tile(f"st{m}")
                for nh in range(2):
                    nc.tensor.matmul(
                        ps[:, nh * NF:(nh + 1) * NF],
                        lhsT=k_ap[ro:ro + 64, m * P:(m + 1) * P],
                        rhs=q_ap[ro:ro + 64, nh * NF:(nh + 1) * NF],
                        start=True, stop=True,
                    )
                e = work.tile([P, N], BF16, name="est", tag="est", bufs=2)
                nc.scalar.activation(e[:], ps[:], AF.Exp)
                pt = work.tile([P, N], BF16, name=f"pt{m}", tag=f"pt{m}", bufs=2)
                # ACT does exp only; 3 of 8 mask-mults go to the otherwise-
                # idle GpSimd engine (SBUF-only op, so Pool may run it)
                eng = nc.gpsimd if m in (0, 2, 4) else nc.vector
                eng.tensor_mul(pt[:], e[:], em_t[m][:])
                pts[h & 1][m] = pt
            if hl >= 0 and m < 4:
                for t in (2 * m, 2 * m + 1):
                    for mc in range(8):
                        nc.tensor.matmul(
                            pv_t[hl][:, t * 64:(t + 1) * 64],
                            lhsT=pts[hl & 1][mc][:, t * P:(t + 1) * P],
                            rhs=v_t[mc][:, hl * 64:(hl + 1) * 64],
                            start=(mc == 0), stop=(mc == 7),
                        )
            if hl >= 0 and 1 <= m <= 4:
                for t in (2 * (m - 1), 2 * (m - 1) + 1):
                    for mc in range(8):
                        nc.tensor.matmul(
                            den8[:, t:t + 1],
                            lhsT=pts[hl & 1][mc][:, t * P:(t + 1) * P],
                            rhs=ones_t[:, 0:1],
                            start=(mc == 0), stop=(mc == 7),
                        )
            if hl >= 0 and m == 5:
                rc = work.tile([P, 8], F32, name=f"rc{hl}", tag="rc", bufs=2)
                nc.vector.reciprocal(rc[:], den8[:])
                rc_t[hl] = rc
            if h >= 2 and m <= 5:
                norm_evict(h - 2, 2 + m)        # tiles 2..7 of head h-2
                if m == 5:
                    del pv_t[h - 2], rc_t[h - 2]
            if hl >= 0 and m >= 6:
                norm_evict(hl, m - 6)           # tiles 0..1 of head h-1
            if h == 0 and m < 4:
                a2_full(0, 4 + m)               # rest of the v vf=0 tiles
            k2, half = (m - 2) // 2, (m - 2) % 2
            its = items_by_head.get(h, ())
            if m >= 2 and k2 < len(its):
                item_chunk(its[k2], half)
        if h == 8:
            # A2 fully emitted; free the wc pool, fetch proj weights there
            wcp_ctx.__exit__(None, None, None)
            late = ctx.enter_context(tc.tile_pool(name="late", bufs=1))
            for c in range(8):
                t = late.tile([P, C], BF16, name=f"pw{c}", tag=f"pw{c}")
                nc.sync.dma_start(t[:], pwT[c * P:(c + 1) * P, :])
                pw_t.append(t)
        if h in tp_sched:
            emit_transposes(tp_sched[h])
    # drain: head 15's remaining evicts (tiles 2..7); the first proj tile's
    # c=0..6 matmuls run UNDER the pair-7 transpose chain (different psum
    # banks, the accumulation group stays pending across it)
    for t in range(2, 8):
        norm_evict(15, t)
    del pv_t[15], rc_t[15]
    ps0 = st_tile("y0")
    for nh in range(2):
        for c in range(7):
            nc.tensor.matmul(
                ps0[:, nh * NF:(nh + 1) * NF],
                lhsT=pw_t[c][:, 0:P],
                rhs=ot[c][:, nh * NF:(nh + 1) * NF],
                start=(c == 0), stop=False,
            )
    emit_transposes(7)

    # ---- output projection ----
    for mt in range(8):
        ps = st_tile(f"y{mt}") if mt else ps0
        for nh in range(2):
            for c in range(0 if mt else 7, 8):
                nc.tensor.matmul(
                    ps[:, nh * NF:(nh + 1) * NF],
                    lhsT=pw_t[c][:, mt * P:(mt + 1) * P],
                    rhs=ot[c][:, nh * NF:(nh + 1) * NF],
                    start=(c == 0), stop=(c == 7),
                )
        for nh in range(2):
            y = work.tile([P, NF], F32, name="y", tag="y", bufs=2)
            nc.scalar.activation(y[:], ps[:, nh * NF:(nh + 1) * NF], AF.Identity,
                                 bias=pb_t[:, mt:mt + 1])
            nc.sync.dma_start(out[mt * P:(mt + 1) * P, nh * NF:(nh + 1) * NF], y[:])


def _prep_inputs(x, attn_mask, qkv_w, proj_w, proj_b):
    """Build the 8 per-core input maps (cores 0-3: branch 0 / x_ori with
    batches 0-3; cores 4-7: branch 1 / x_v)."""
    bf = ml_dtypes.bfloat16
    scale = D ** (-0.5)
    q_w, k_w, v_w = qkv_w[0:C], qkv_w[C:2 * C], qkv_w[2 * C:3 * C]
    wT_br = [
        np.ascontiguousarray(np.vstack([q_w * scale, k_w, v_w]).T.astype(bf)),
        np.ascontiguousarray(np.vstack([v_w * scale, v_w, v_w]).T.astype(bf)),
    ]
    em_br = [np.ascontiguousarray(np.exp(attn_mask[br, 0]).T.astype(bf))
             for br in range(2)]
    pwT = np.ascontiguousarray(proj_w.T.astype(bf))
    pb = np.ascontiguousarray(proj_b.astype(np.float32))
    ident = np.eye(P, dtype=np.float32)
    in_maps = []
    for core in range(8):
        br, b = core // 4, core % 4
        in_maps.append({
            "xT": np.ascontiguousarray(x[b].T.astype(bf)),
            "wT": wT_br[br],
            "em": em_br[br],
            "pwT": pwT,
            "pb": pb,
            "ident": ident,
        })
    return in_maps


def _run(inputs, trace=False, **kw):
    global _nc_cache
    from concourse.bass_utils import run_bass_kernel_spmd
    if _nc_cache is None:
        _nc_cache = _build()
    in_maps = _prep_inputs(**inputs)
    res = run_bass_kernel_spmd(_nc_cache, in_maps, core_ids=list(range(8)),
                               trace=trace, **kw)
    outs = [np.asarray(res.results[i]["out"], dtype=np.float32).T
            for i in range(8)]
    x_ori = np.stack(outs[0:4])
    x_v = np.stack(outs[4:8])
    return (x_v, x_ori), res


def kernel(x, attn_mask, qkv_w, proj_w, proj_b):
    (x_v, x_ori), _ = _run(dict(x=np.asarray(x), attn_mask=np.asarray(attn_mask),
                                qkv_w=np.asarray(qkv_w), proj_w=np.asarray(proj_w),
                                proj_b=np.asarray(proj_b)))
    return (x_v, x_ori)
